# revision 12
# baseline (speedup 1.0000x reference)
"""Trainium2 Bass kernel: per-superpixel mean of CNN features + linear head.

reference computes:
    sums[s, f]  = segment_sum(features, superpixel)      # 1024 segments
    out[s, c]   = (sums[s] / max(count_s, 1)) @ w_node.T # [1024, 21]

Algebraic restructure: project each pixel's 256-dim feature to the 22-dim
padded class space FIRST, then segment-sum the projections:
    out[s, c] = segsum(feats @ w_aug.T)[s, c] / count_s
The segment reduction is a [pix,22].T @ onehot[pix,1024] matmul per
128-pixel tile, accumulated in PSUM across tiles.

v2 layout choices (vs v1):
  * features are transposed to [256 f, pix] bf16 on the host, so the
    projection reads fT blocks directly as the PE stationary operand —
    no per-tile PE transposes, fast (FWL) weight loads, and half the
    HBM traffic of fp32.
  * superpixel labels and the iota row are encoded as distinct bf16 BIT
    PATTERNS (0x4000+v) so the onehot is_equal compare runs all-bf16
    (DVE 4x mode eligible) yet stays exact.
  * each tile's two 512-wide segment-sum matmuls go to DIFFERENT PE
    column groups (tile parity rotates over 4 groups), so their moving
    streams overlap via separate XBUSes.

Sharding: 512*512 pixels split evenly across 8 cores (segment-sum is
permutation-invariant). Each core emits a [128, 512] partial holding 4
groups x 22 class rows; the host adds the partials, divides by counts
(np.bincount) and transposes.
"""

import os as _os

import numpy as np
import ml_dtypes

import concourse.mybir as mybir
import concourse.tile as tile
from concourse import bacc
from concourse.bass_utils import run_bass_kernel_spmd

N_CORES = 8
P = 128
F = 256                      # feature dim
NUM_SP = 1024                # superpixel labels
C = 21                       # classes
CP = 22                      # classes padded even
NPIX = 512 * 512
PIX_PER_CORE = NPIX // N_CORES       # 32768
N_TILES = PIX_PER_CORE // P          # 256

CHUNK_PIX = int(_os.environ.get("KERNEL_CHUNK_PIX", "2048"))
N_CHUNKS = PIX_PER_CORE // CHUNK_PIX
TILES_PER_CHUNK = CHUNK_PIX // P

F32 = mybir.dt.float32
BF16 = mybir.dt.bfloat16


def _build_nc():
    work_bufs = int(_os.environ.get("KERNEL_WORK_BUFS", "6"))
    psum_bufs = int(_os.environ.get("KERNEL_PSUM_BUFS", "3"))
    chunk_bufs = int(_os.environ.get("KERNEL_CHUNK_BUFS", "3"))
    split_first = bool(int(_os.environ.get("KERNEL_SPLIT_FIRST", "1")))
    gp_every = int(_os.environ.get("KERNEL_GP_EVERY", "4"))
    gp_mode = _os.environ.get("KERNEL_GP_MODE", "ls")

    nc = bacc.Bacc("TRN2", target_bir_lowering=False)

    feats = nc.dram_tensor(
        "feats", [N_CHUNKS, 2, P, CHUNK_PIX], BF16, kind="ExternalInput"
    )
    labels = nc.dram_tensor("labels", [P, N_TILES], F32, kind="ExternalInput")
    labels16 = nc.dram_tensor(
        "labels16", [P, N_TILES, 2], mybir.dt.int16, kind="ExternalInput"
    )
    iota = nc.dram_tensor("iota", [P, NUM_SP], BF16, kind="ExternalInput")
    w_aug = nc.dram_tensor("w_aug", [P, 2, CP], BF16, kind="ExternalInput")
    out = nc.dram_tensor("out", [P, 512], F32, kind="ExternalOutput")

    with tile.TileContext(nc) as tc:
        with (
            tc.tile_pool(name="const", bufs=1) as const_pool,
            tc.tile_pool(name="chunk", bufs=chunk_bufs) as chunk_pool,
            tc.tile_pool(name="work", bufs=work_bufs) as work_pool,
            tc.tile_pool(name="psum", bufs=psum_bufs, space="PSUM") as psum_pool,
            tc.tile_pool(name="accp", bufs=1, space="PSUM") as acc_pool,
        ):
            iota_sb = const_pool.tile([P, NUM_SP], BF16)
            nc.sync.dma_start(out=iota_sb[:], in_=iota[:])
            labels_sb = const_pool.tile([P, N_TILES], F32)
            nc.sync.dma_start(out=labels_sb[:], in_=labels[:])
            w_sb = const_pool.tile([P, 2, CP], BF16)
            nc.sync.dma_start(out=w_sb[:], in_=w_aug[:])
            if gp_every:
                labels16_sb = const_pool.tile([P, N_TILES, 2], mybir.dt.int16)
                nc.sync.dma_start(out=labels16_sb[:], in_=labels16[:])
                ones_sb = const_pool.tile([P, 2], BF16)
                nc.gpsimd.memset(ones_sb[:], 1.0)

            # persistent accumulator: group g = 2*(tg%2)+half accumulates
            # class sums for sp half `half` into partitions [32g, 32g+CP)
            acc = acc_pool.tile([P, 512], F32)

            for c in range(N_CHUNKS):
                feats_sb = chunk_pool.tile([P, 2, CHUNK_PIX], BF16, tag="feats")
                for h in range(2):
                    if c == 0 and split_first:
                        q = CHUNK_PIX // 2
                        for k in range(2):
                            nc.sync.dma_start(
                                out=feats_sb[:, h, k * q : (k + 1) * q],
                                in_=feats[c, h][:, k * q : (k + 1) * q],
                            )
                    else:
                        nc.sync.dma_start(out=feats_sb[:, h, :], in_=feats[c, h])

                for t in range(TILES_PER_CHUNK):
                    tg = c * TILES_PER_CHUNK + t
                    col = t * P

                    # proj[pix, c] = sum_f fT[f, pix] * w_aug[f, c]
                    proj_ps = psum_pool.tile([P, CP], F32, tag="projps")
                    nc.tensor.matmul(
                        out=proj_ps[:],
                        lhsT=feats_sb[:, 0, col : col + P],
                        rhs=w_sb[:, 0, :],
                        start=True,
                        stop=False,
                    )
                    nc.tensor.matmul(
                        out=proj_ps[:],
                        lhsT=feats_sb[:, 1, col : col + P],
                        rhs=w_sb[:, 1, :],
                        start=False,
                        stop=True,
                    )
                    pq_sb = work_pool.tile([P, CP], BF16, tag="pqsb")
                    nc.scalar.activation(
                        out=pq_sb[:],
                        in_=proj_ps[:],
                        func=mybir.ActivationFunctionType.Copy,
                    )

                    # onehot[p, s] = (iota[p, s] == label[p]); all-bf16
                    # bit-pattern compare (exact). Every gp_every-th tile is
                    # offloaded to the otherwise-idle GPSIMD engine.
                    onehot = work_pool.tile([P, NUM_SP], BF16, tag="onehot")
                    if gp_every and tg % gp_every == gp_every - 1:
                        if gp_mode == "ls":
                            nc.gpsimd.local_scatter(
                                out_ap=onehot[:],
                                data_ap=ones_sb[:],
                                idxs_ap=labels16_sb[:, tg, :],
                                channels=P,
                                num_elems=NUM_SP,
                                num_idxs=2,
                            )
                        else:
                            nc.gpsimd.tensor_scalar(
                                onehot[:],
                                iota_sb[:],
                                labels_sb[:, tg : tg + 1],
                                None,
                                mybir.AluOpType.is_equal,
                            )
                    else:
                        nc.vector.tensor_scalar(
                            onehot[:],
                            iota_sb[:],
                            labels_sb[:, tg : tg + 1],
                            None,
                            mybir.AluOpType.is_equal,
                        )

                    # acc[32g + c, s'] += pq[pix, c] * onehot[pix, 512h + s']
                    for half in range(2):
                        g = 2 * (tg % 2) + half
                        row = 32 * g
                        nc.tensor.matmul(
                            out=acc[row : row + CP, :],
                            lhsT=pq_sb[:],
                            rhs=onehot[:, 512 * half : 512 * (half + 1)],
                            start=tg < 2,
                            stop=tg >= N_TILES - 2,
                            tile_position=(0, row),
                            skip_group_check=True,
                        )

            out_sb = chunk_pool.tile([P, 512], F32, tag="outsb")
            nc.scalar.activation(
                out=out_sb[:], in_=acc[:], func=mybir.ActivationFunctionType.Copy
            )
            nc.sync.dma_start(out=out[:], in_=out_sb[:])

    nc.compile()
    return nc


def _install_ntff_hook():
    """Register the axon NTFF profiling hook when the image's antenv
    lacks axon_hooks (mirrors trn_agent_boot._ntff_profile_via_ctypes)."""
    import contextlib
    import ctypes
    import sys
    import types

    if "antenv.axon_hooks" in sys.modules:
        return
    lib = ctypes.CDLL("/opt/axon/libaxon_pjrt.so")
    if not hasattr(lib, "axon_start_nrt_profile"):
        return
    lib.axon_start_nrt_profile.argtypes = [
        ctypes.POINTER(ctypes.c_int64),
        ctypes.c_size_t,
    ]
    lib.axon_start_nrt_profile.restype = ctypes.c_int64
    lib.axon_stop_nrt_profile.argtypes = [ctypes.c_char_p]
    lib.axon_stop_nrt_profile.restype = ctypes.c_int64

    @contextlib.contextmanager
    def _hook(output_dir, device_ids):
        import jax

        jax.devices()
        if device_ids:
            ids = (ctypes.c_int64 * len(device_ids))(*device_ids)
            rc = lib.axon_start_nrt_profile(ids, len(device_ids))
        else:
            rc = lib.axon_start_nrt_profile(None, 0)
        if rc != 0:
            raise RuntimeError(f"axon_start_nrt_profile rc={rc}")
        try:
            yield
        finally:
            n = lib.axon_stop_nrt_profile(str(output_dir).encode())
            print(f"profile: {n} file(s) written to {output_dir}", file=sys.stderr)

    mod = types.ModuleType("antenv.axon_hooks")
    mod.get_axon_ntff_profile_hook = lambda: _hook
    mod.set_axon_ntff_profile_hook = lambda h: None
    sys.modules["antenv.axon_hooks"] = mod


_NC_CACHE = None


def _get_nc():
    global _NC_CACHE
    if _NC_CACHE is None:
        _NC_CACHE = _build_nc()
    return _NC_CACHE


def _encode_bf16_pattern(v):
    """Map small non-negative ints to distinct, exactly-comparable bf16
    bit patterns (0x4000 + v are all normal, distinct values)."""
    return (0x4000 + np.asarray(v, dtype=np.uint16)).view(ml_dtypes.bfloat16)


def kernel(features, superpixel, w_node):
    features = np.asarray(features, dtype=np.float32)
    superpixel = np.asarray(superpixel)
    w_node = np.asarray(w_node, dtype=np.float32)

    feats_flat = features.reshape(NPIX, F)
    sp_flat = superpixel.reshape(NPIX).astype(np.int64)

    # host-side layout: transposed bf16 features [256 f, NPIX pix]
    fT = np.ascontiguousarray(feats_flat.astype(ml_dtypes.bfloat16).T)
    enc = _encode_bf16_pattern(sp_flat)

    wa = np.zeros((F, CP), dtype=np.float32)
    wa[:, :C] = w_node.T
    # w_aug dram layout [P, 2, CP]: [f_lo, h, c] = w_aug[128h + f_lo, c]
    wa_bf = np.ascontiguousarray(
        wa.astype(ml_dtypes.bfloat16).reshape(2, P, CP).transpose(1, 0, 2)
    )
    iota = np.ascontiguousarray(
        np.broadcast_to(_encode_bf16_pattern(np.arange(NUM_SP))[None, :], (P, NUM_SP))
    )

    in_maps = []
    for core in range(N_CORES):
        lo = core * PIX_PER_CORE
        fc = fT[:, lo : lo + PIX_PER_CORE]
        # feats[c, h, f, j] = fT[128h + f, lo + c*CHUNK_PIX + j]
        f_dev = np.ascontiguousarray(
            fc.reshape(2, P, N_CHUNKS, CHUNK_PIX).transpose(2, 0, 1, 3)
        )
        # labels[p, tg] = enc(sp[lo + 128*tg + p]); scalar port is fp32,
        # bf16->fp32 is exact so the pattern compare still matches
        lab_core = enc[lo : lo + PIX_PER_CORE].reshape(N_TILES, P).T
        lab = np.ascontiguousarray(lab_core.astype(np.float32))
        # int16 raw labels + trailing -1 (ignored) for gpsimd local_scatter
        lab16 = np.full((P, N_TILES, 2), -1, dtype=np.int16)
        lab16[:, :, 0] = (
            sp_flat[lo : lo + PIX_PER_CORE].reshape(N_TILES, P).T.astype(np.int16)
        )
        in_maps.append(
            {
                "feats": f_dev,
                "labels": lab,
                "labels16": lab16,
                "iota": iota,
                "w_aug": wa_bf,
            }
        )

    trace = bool(int(_os.environ.get("KERNEL_TRACE", "0")))
    repeat = int(_os.environ.get("KERNEL_REPEAT", "1"))
    kwargs = {}
    if trace:
        _install_ntff_hook()
        import concourse.bass_utils as _bu

        _bu.upload_artifacts = lambda tmpdir: tmpdir
    base_dir = _os.environ.get("KERNEL_TRACE_DIR") or None
    for rep in range(repeat):
        if trace and base_dir:
            kwargs["tmpdir"] = _os.path.join(base_dir, f"rep{rep}")
            _os.makedirs(kwargs["tmpdir"], exist_ok=True)
        res = run_bass_kernel_spmd(
            _get_nc(), in_maps, core_ids=list(range(N_CORES)), trace=trace, **kwargs
        )
        if trace:
            print(f"HW exec time: {res.exec_time_ns} ns")
            print(f"profile_json: {res.profile_json}")

    total = np.zeros((C, NUM_SP), dtype=np.float64)
    for r in res.results:
        o = np.asarray(r["out"], dtype=np.float64)
        total[:, 0:512] += o[0:C] + o[64 : 64 + C]
        total[:, 512:1024] += o[32 : 32 + C] + o[96 : 96 + C]
    counts = np.bincount(sp_flat, minlength=NUM_SP).astype(np.float64)
    node_potentials = total / np.clip(counts, 1.0, None)
    return np.ascontiguousarray(node_potentials.T).astype(np.float32)
